# revision 5
# baseline (speedup 1.0000x reference)
"""Grok1-style MoE (T=2048, H=1024, E=8, I=2048, top-2) on 8 Trainium2 cores.

Strategy (expert-parallel, per the sharding hint):
  - Host: compute the tiny router (x @ gate_w, tanh softcap, top-2, softmax)
    and dispatch tokens by expert assignment (the "all-to-all dispatch" step:
    with full inputs on the host, dispatch = gather per expert), packing the
    per-core shards in a device-friendly tiled layout, cast to bf16.
  - Device (SPMD, 1 expert per core): grouped GEMM
        gT = wg_e^T x_e^T ; uT = wu_e^T x_e^T   (computed transposed, [I, M])
        act = gelu_tanh(gT) * uT                ([I, M], bf16)
        yT  = wd_e^T @ act                      ([H, M], tokens stay on the
                                                 moving dim -> no 128-token
                                                 padding anywhere)
  - Host: combine = scatter-add prob_e * y_e^T into [T, H] (the "all-to-all
    combine weighted by router probs").

All matmuls run in bf16 (1 col/cycle on the 128x128 PE, same rate as fp32r,
but half the HBM traffic and 4x faster weight loads via FWL), accumulating
in fp32 PSUM.
"""

import numpy as np
import ml_dtypes

import concourse.mybir as mybir
import concourse.tile as tile
from concourse import bacc
from concourse.bass_utils import run_bass_kernel_spmd

T, H, E, I_DIM, TOPK = 2048, 1024, 8, 2048, 2
SOFTCAP = 30.0
P = 128
N_CORES = 8
KH = H // P       # 8 contraction tiles (phase 1)
NI = I_DIM // P   # 16 i tiles
NHT = H // P      # 8 output h tiles (phase 2)

BF16 = ml_dtypes.bfloat16

_compiled = {}
LAST_RESULTS = None


def _m_chunks(M_PAD):
    """Split [0, M_PAD) into near-equal chunks <= 512 (multiples of 4)."""
    n_chunks = max(1, -(-M_PAD // 512))
    base = M_PAD // n_chunks
    base -= base % 4
    chunks, off = [], 0
    for c in range(n_chunks):
        ln = base if c < n_chunks - 1 else M_PAD - off
        chunks.append((off, ln))
        off += ln
    return chunks


def _build(M_PAD):
    f32 = mybir.dt.float32
    bf16 = mybir.dt.bfloat16
    chunks = _m_chunks(M_PAD)

    nc = bacc.Bacc("TRN2", target_bir_lowering=False, num_devices=N_CORES)
    # Host-packed layouts (all DMAs contiguous per partition):
    #   xt  [KH, P, M]        : xt[k, p, m] = x_e[m, k*P+p]
    #   wgu [NI, 2, P, KH*P]  : wgu[it, 0, p, k*P+i] = wg_e[k*P+p, it*P+i]
    #                           wgu[it, 1, ...] same for wu_e
    #   wdp [NHT, P, NI*P]    : wdp[ht, p, it*P+h] = wd_e[it*P+p, ht*P+h]
    xt = nc.dram_tensor("xt", [KH, P, M_PAD], bf16, kind="ExternalInput")
    wgu = nc.dram_tensor("wgu", [NI, 2, P, KH * P], bf16, kind="ExternalInput")
    wdp = nc.dram_tensor("wdp", [NHT, P, NI * P], bf16, kind="ExternalInput")
    y = nc.dram_tensor("y", [NHT * P, M_PAD], f32, kind="ExternalOutput")

    with tile.TileContext(nc) as tc:
        with (
            tc.tile_pool(name="persist", bufs=1) as persist,
            tc.tile_pool(name="wtiles", bufs=6) as wtiles,
            tc.tile_pool(name="outs", bufs=2) as outs,
            tc.tile_pool(name="psum", bufs=2, space="PSUM") as psum,
        ):
            xt_sb = persist.tile([P, KH, M_PAD], bf16)
            acts = persist.tile([P, NI, M_PAD], bf16)
            wd_sb = persist.tile([P, NHT, NI * P], bf16)
            zero = persist.tile([P, P], bf16)

            def wgu_src(it):
                return wgu.ap()[it].rearrange("g p (k i) -> p g k i", i=P)

            # PE warm-up: the HAM clock gate starts at 1.2 GHz and only
            # releases after ~3.4us of sustained activity.  Burn part of the
            # initial DMA wait on dummy matmuls; the first real matmuls then
            # run cold-but-busy through the DMA ramp (which keeps HAM's
            # activity window filled) and the stream is warm from ~3.4us in.
            nc.vector.memset(zero[:], 0.0)
            warm_ps = psum.tile([P, P], f32, tag="warm")
            for _ in range(16):
                nc.tensor.matmul(warm_ps[:], zero[:], zero[:], start=True, stop=True)

            # Startup feed, both HWDGE rings: xt per k-tile (first matmul only
            # needs k=0), first wgu tile split in k-halves.  Critical pieces
            # go on the sync ring: scalar's first descriptor is delayed
            # ~1.3us by its ACT_TABLE_LOAD.
            wgu_sbs = {}
            wgu_sbs[0] = wtiles.tile([P, 2, KH, P], bf16, tag="wgu", name="wgu0")
            nc.sync.dma_start(wgu_sbs[0][:, :, : KH // 2], wgu_src(0)[:, :, : KH // 2])
            nc.sync.dma_start(xt_sb[:, 0], xt.ap()[0])
            nc.scalar.dma_start(xt_sb[:, 1], xt.ap()[1])
            nc.scalar.dma_start(wgu_sbs[0][:, :, KH // 2 :], wgu_src(0)[:, :, KH // 2 :])
            for k in range(2, KH):
                eng = nc.sync if k % 2 == 0 else nc.scalar
                eng.dma_start(xt_sb[:, k], xt.ap()[k])

            def load_wgu(it):
                if it < NI and it not in wgu_sbs:
                    wgu_sbs[it] = wtiles.tile(
                        [P, 2, KH, P], bf16, tag="wgu", name=f"wgu{it}"
                    )
                    eng = nc.sync if it % 2 == 0 else nc.scalar
                    eng.dma_start(wgu_sbs[it][:], wgu_src(it))

            # Phase 1: gT/uT = wg^T xT / wu^T xT per i-tile; act = gelu(g)*u.
            # wd (consumed only in phase 2) streams during the back half.
            for it in range(NI):
                load_wgu(it)
                load_wgu(it + 1)
                load_wgu(it + 2)
                wgu_sb = wgu_sbs.pop(it)

                for (m0, ml) in chunks:
                    g_ps = psum.tile([P, ml], f32, tag="g")
                    u_ps = psum.tile([P, ml], f32, tag="u")
                    for k in range(KH):
                        nc.tensor.matmul(
                            g_ps[:],
                            wgu_sb[:, 0, k],
                            xt_sb[:, k, m0 : m0 + ml],
                            start=(k == 0),
                            stop=(k == KH - 1),
                        )
                    for k in range(KH):
                        nc.tensor.matmul(
                            u_ps[:],
                            wgu_sb[:, 1, k],
                            xt_sb[:, k, m0 : m0 + ml],
                            start=(k == 0),
                            stop=(k == KH - 1),
                        )
                    nc.scalar.activation(
                        acts[:, it, m0 : m0 + ml], g_ps[:],
                        mybir.ActivationFunctionType.Gelu_apprx_tanh,
                    )
                    nc.vector.tensor_mul(
                        acts[:, it, m0 : m0 + ml], acts[:, it, m0 : m0 + ml], u_ps[:]
                    )
                if it >= NI - NHT:
                    ht = it - (NI - NHT)
                    eng = nc.sync if it % 2 == 0 else nc.scalar
                    eng.dma_start(wd_sb[:, ht], wdp.ap()[ht])

            # Phase 2: yT[h, m] = sum_i wd[i, h] * act[i, m]  (tokens moving)
            for ht in range(NHT):
                for (m0, ml) in chunks:
                    d_ps = psum.tile([P, ml], f32, tag="d")
                    for it in range(NI):
                        nc.tensor.matmul(
                            d_ps[:],
                            wd_sb[:, ht, it * P : (it + 1) * P],
                            acts[:, it, m0 : m0 + ml],
                            start=(it == 0),
                            stop=(it == NI - 1),
                        )
                    y_sb = outs.tile([P, ml], f32, tag="y")
                    nc.scalar.activation(
                        y_sb[:], d_ps[:],
                        mybir.ActivationFunctionType.Copy,
                    )
                    nc.sync.dma_start(
                        y.ap()[ht * P : (ht + 1) * P, m0 : m0 + ml], y_sb[:]
                    )

    nc.compile()
    return nc


def kernel(hidden_states, gate_w, wg, wu, wd):
    global LAST_RESULTS
    x = np.ascontiguousarray(np.asarray(hidden_states, dtype=np.float32))
    gw = np.asarray(gate_w, dtype=np.float32)
    wg = np.asarray(wg, dtype=np.float32)
    wu = np.asarray(wu, dtype=np.float32)
    wd = np.asarray(wd, dtype=np.float32)

    # Router on host (part of the dispatch/sharding step).
    logits = np.tanh((x @ gw) / np.float32(SOFTCAP))
    top2 = np.argsort(-logits, axis=1, kind="stable")[:, :TOPK]  # [T, 2]
    v = np.take_along_axis(logits, top2, axis=1)                 # descending
    ex = np.exp(v - v[:, :1])
    pk = (ex / ex.sum(axis=1, keepdims=True)).astype(np.float32)  # [T, 2]

    token_ids, probs_e = [], []
    for e in range(E):
        mask = top2 == e
        rows = np.where(mask.any(axis=1))[0]
        kk = np.argmax(mask[rows], axis=1)
        token_ids.append(rows)
        probs_e.append(pk[rows, kk])

    n_max = max(len(r) for r in token_ids)
    M_PAD = max(256, -(-n_max // 8) * 8)

    nc = _compiled.get(M_PAD)
    if nc is None:
        nc = _build(M_PAD)
        _compiled[M_PAD] = nc

    x_bf = x.astype(BF16)
    wg_bf = wg.astype(BF16)
    wu_bf = wu.astype(BF16)
    wd_bf = wd.astype(BF16)

    in_maps = []
    for e in range(E):
        ids = token_ids[e]
        xe = np.zeros((M_PAD, H), BF16)
        xe[: len(ids)] = x_bf[ids]
        # [M_PAD, KH, P] -> [KH, P, M_PAD]
        xt_e = np.ascontiguousarray(xe.reshape(M_PAD, KH, P).transpose(1, 2, 0))
        # [H, I] -> [NI, P, KH*P]
        def pack_w(w_e):
            w4 = w_e.reshape(KH, P, NI, P)
            return w4.transpose(2, 1, 0, 3).reshape(NI, P, KH * P)
        wgu_e = np.ascontiguousarray(
            np.stack([pack_w(wg_bf[e]), pack_w(wu_bf[e])], axis=1)
        )
        # [I, H] -> [NHT, P, NI*P]
        wdp_e = np.ascontiguousarray(
            wd_bf[e].reshape(NI, P, NHT, P).transpose(2, 1, 0, 3).reshape(
                NHT, P, NI * P
            )
        )
        in_maps.append({"xt": xt_e, "wgu": wgu_e, "wdp": wdp_e})

    res = run_bass_kernel_spmd(nc, in_maps, core_ids=list(range(N_CORES)))
    LAST_RESULTS = res

    out = np.zeros((T, H), np.float32)
    for e in range(E):
        ids = token_ids[e]
        yt = res.results[e]["y"][:, : len(ids)]                  # [H, n]
        out[ids] += (yt * probs_e[e][None, :]).T
    return out
